# revision 1
# baseline (speedup 1.0000x reference)
"""DiT block (final) Trainium2 Bass kernel — data-parallel over batch.

Core b computes batch element b end-to-end (B=8 == n_cores), no collectives.
All GEMMs run on the PE in bf16 (1 cyc/row) with fp32 PSUM accumulation;
weights are transposed into [in_feat(part), out_feat] layout on the fly via
gpsimd cast-DMA (f32->bf16) + HWDGE DMA-transpose.  Heads (72 wide) are
stored packed in 128-row tiles; per-head operands are unpacked/repacked with
shifted-diagonal selector matmuls on the PE.  All PSUM tiles are one bank
([<=128, <=512] fp32) drawn from a single 7-slot pool so independent
accumulation chains pipeline across heads/tiles.
"""

import contextlib

import numpy as np

import concourse.bass as bass
import concourse.tile as tile
import concourse.mybir as mybir
from concourse.masks import make_identity
from concourse.vector_clock import ScopedClock

F32 = mybir.dt.float32
BF16 = mybir.dt.bfloat16
AF = mybir.ActivationFunctionType

B = 8
N = 1024
H = 1152
NH = 16
HD = 72
CTXL = 77
CTXD = 768
MLP = 4608
EPS = 1e-6
P = 128
NT = N // P       # 8
NI = H // P       # 9
NIC = CTXD // P   # 6
NJ = MLP // P     # 36
ATT_SCALE = HD ** -0.5

SH_MSA, SC_MSA, G_MSA = 0, H, 2 * H
SH_MLP, SC_MLP, G_MLP = 3 * H, 4 * H, 5 * H

# whole-head output chunks for v projections: (f0, width, first head, n heads)
VCH = [(0, 360, 0, 5), (360, 360, 5, 5), (720, 360, 10, 5), (1080, 72, 15, 1)]


class SplitDrainTileContext(tile.TileContext):
    """This walrus build rejects a Drain carrying >1 sem wait; split the
    end-of-kernel drain into one drain per wait."""

    def _drain_and_barrier(self, tick_clock, wait_clock):
        nc = self.nc
        drain_bi = nc.sync.drain()
        wait_clock.add_sem_waits(
            drain_bi.ins, ScopedClock({None: tick_clock.global_clock})
        )
        si = drain_bi.ins.sync_info
        waits = list(si.on_wait or [])
        if len(waits) > 1:
            del si.on_wait[1:]
            for w in waits[1:]:
                d2 = nc.sync.drain()
                d2.ins.sync_info = mybir.SyncInfo(on_wait=[w], on_update=[])
        nc.all_engine_barrier()
        popped = nc._tile_sem_poison_stack.pop()
        assert popped is self._sem_poison
        nc.clear_and_free_semaphores(list(self.sems.allocated().values()))
        nc.all_engine_barrier()


def _split_waits(nc, limit=1):
    """This walrus build supports very few sync-wait slots per instruction.
    Move extra waits onto same-engine NOPs inserted just before each
    instruction (in-order engines make this semantics-preserving)."""
    n_new = 0
    for f in nc.m.functions:
        for blk in f.blocks:
            insts = list(blk.instructions)
            out = []
            changed = False
            for inst in insts:
                si = getattr(inst, "sync_info", None)
                eng = getattr(inst, "engine", None)
                if (si and si.on_wait and len(si.on_wait) > limit
                        and eng is not None
                        and eng != mybir.EngineType.Unassigned):
                    waits = list(si.on_wait)
                    del si.on_wait[:]
                    si.on_wait.extend(waits[-limit:])
                    for w in waits[:-limit]:
                        n_new += 1
                        nop = mybir.InstNoOp(
                            name=f"wsplit-{n_new}-{inst.name}", ins=[], outs=[])
                        nop.engine = eng
                        nop.sync_info = mybir.SyncInfo(on_wait=[w], on_update=[])
                        out.append(nop)
                    changed = True
                out.append(inst)
            if changed:
                blk.instructions[:] = out
    return n_new


def _head_tiles():
    """Head h spans packed rows [72h, 72h+72): list of (it, shift) where
    shift = 72h - 128it; packed tile `it` row r holds head dim r - shift."""
    out = []
    for h in range(NH):
        lo, hi = HD * h, HD * (h + 1)
        out.append([(it, HD * h - P * it)
                    for it in range(lo // P, (hi - 1) // P + 1)])
    return out


HEAD_TILES = _head_tiles()
TILE_HEADS = [[] for _ in range(NI)]
for _h, _tiles in enumerate(HEAD_TILES):
    for _it, _s in _tiles:
        TILE_HEADS[_it].append((_h, _s))


def build(nc: bass.Bass, split_waits=True):
    x_d = nc.dram_tensor("x", [N, H], F32, kind="ExternalInput")
    c_d = nc.dram_tensor("c", [CTXL, CTXD], F32, kind="ExternalInput")
    temb_d = nc.dram_tensor("t_emb", [1, H], F32, kind="ExternalInput")
    qkvw_d = nc.dram_tensor("qkv_w", [3 * H, H], F32, kind="ExternalInput")
    qkvb_d = nc.dram_tensor("qkv_b", [3 * H], F32, kind="ExternalInput")
    projw_d = nc.dram_tensor("proj_w", [H, H], F32, kind="ExternalInput")
    projb_d = nc.dram_tensor("proj_b", [H], F32, kind="ExternalInput")
    tqw_d = nc.dram_tensor("to_q_w", [H, H], F32, kind="ExternalInput")
    tkw_d = nc.dram_tensor("to_k_w", [H, CTXD], F32, kind="ExternalInput")
    tvw_d = nc.dram_tensor("to_v_w", [H, CTXD], F32, kind="ExternalInput")
    tow_d = nc.dram_tensor("to_out_w", [H, H], F32, kind="ExternalInput")
    tob_d = nc.dram_tensor("to_out_b", [H], F32, kind="ExternalInput")
    w1_d = nc.dram_tensor("mlp_w1", [MLP, H], F32, kind="ExternalInput")
    b1_d = nc.dram_tensor("mlp_b1", [MLP], F32, kind="ExternalInput")
    w2_d = nc.dram_tensor("mlp_w2", [H, MLP], F32, kind="ExternalInput")
    b2_d = nc.dram_tensor("mlp_b2", [H], F32, kind="ExternalInput")
    adaw_d = nc.dram_tensor("ada_w", [6 * H, H], F32, kind="ExternalInput")
    adab_d = nc.dram_tensor("ada_b", [6 * H], F32, kind="ExternalInput")
    out_d = nc.dram_tensor("out", [N, H], F32, kind="ExternalOutput")
    x3_d = nc.dram_tensor("x3_scratch", [N, H], F32)

    with SplitDrainTileContext(nc) as tc, contextlib.ExitStack() as ctx:
        EC = ctx.enter_context
        const = EC(tc.tile_pool(name="const", bufs=1))
        persist = EC(tc.tile_pool(name="persist", bufs=1))
        work = EC(tc.tile_pool(name="work", bufs=3))              # tiny scratch
        psF = EC(tc.tile_pool(name="psF", bufs=8, space="PSUM"))  # 1-bank slots

        def psum(parts=P, width=512):
            return psF.tile([parts, width], F32, tag="f", name="ps")

        identb = const.tile([P, P], BF16)
        make_identity(nc, identb[:])
        identf = const.tile([P, P], F32)
        make_identity(nc, identf[:])
        ones_row = const.tile([1, 512], BF16)
        nc.vector.memset(ones_row[:], 1.0)
        eps_col = const.tile([P, 1], F32)
        nc.vector.memset(eps_col[:], EPS)
        # shifted diagonal: sel[d, c] = 1 iff c == d + 128
        sel = const.tile([P, 328], BF16)
        nc.gpsimd.memset(sel[:], 0.0)
        nc.gpsimd.affine_select(
            out=sel[:], in_=sel[:], compare_op=mybir.AluOpType.not_equal,
            fill=1.0, base=128, pattern=[[-1, 328]], channel_multiplier=1)

        mod_sb = persist.tile([P, 6 * H], BF16)

        def wT_load(pool, dst, w_dram, o0, o_len, tag="wnat"):
            """dst[:, :, s*128...] <- W.T columns for W rows [o0, o0+o_len)."""
            i_len = w_dram.shape[1]
            for s in range(o_len // P):
                so = o0 + s * P
                nat = pool.tile([P, i_len], BF16, tag=f"{tag}{i_len}", bufs=3)
                nc.gpsimd.dma_start(nat[:], w_dram[so:so + P, :])
                nc.sync.dma_start(
                    out=dst[:, :, s * P:(s + 1) * P], in_=nat[:], transpose=True)

        tb_state = [0]

        def transpose_block(dst_bf, src, f32=False):
            p = src.shape[0]
            ident = identf if f32 else identb
            pt = psF.tile([P, P + P], F32 if f32 else BF16, tag="f", name="pt")
            nc.tensor.transpose(pt[:, :p], src, ident[0:p, 0:p])
            tb_state[0] ^= 1
            if tb_state[0]:
                nc.vector.tensor_copy(dst_bf, pt[:, :p])
            else:
                nc.scalar.activation(dst_bf, pt[:, :p], AF.Identity)

        # =====================================================
        # Phase 0: adaLN modulation table (pre-broadcast over partitions)
        # =====================================================
        temb_sb = work.tile([1, H], F32, tag="temb", bufs=1)
        nc.sync.dma_start(temb_sb[:], temb_d[:])
        s_bf = work.tile([1, H], BF16, tag="sbf", bufs=1)
        nc.scalar.activation(s_bf[:], temb_sb[:], AF.Silu)

        with tc.tile_pool(name="p0", bufs=1) as p0, \
             tc.tile_pool(name="p0w", bufs=2) as p0w:
            sbc_sb = p0.tile([P, H], BF16)
            for f0 in range(0, H, 512):
                w = min(512, H - f0)
                sp = psum()
                nc.tensor.matmul(sp[:, :w], ones_row[:, :P],
                                 s_bf[:, f0:f0 + w], start=True, stop=True)
                nc.vector.tensor_copy(sbc_sb[:, f0:f0 + w], sp[:, :w])
            s_rep = p0.tile([P, NI, P], BF16)
            for it in range(NI):
                transpose_block(s_rep[:, it, :], sbc_sb[:, it * P:(it + 1) * P])

            adab_bf = p0.tile([1, 6 * H], BF16)
            nc.gpsimd.dma_start(adab_bf[:], adab_d[None, :])
            ACH = 384
            for o0 in range(0, 6 * H, ACH):
                adat = p0w.tile([P, NI, ACH], BF16, tag="adat")
                wT_load(p0w, adat, adaw_d, o0, ACH)
                mp = psum()
                for it in range(NI):
                    nc.tensor.matmul(mp[:, :ACH], s_rep[:, it, :],
                                     adat[:, it, :], start=(it == 0), stop=False)
                nc.tensor.matmul(mp[:, :ACH], ones_row[:, :P],
                                 adab_bf[:, o0:o0 + ACH], start=False, stop=True)
                nc.vector.tensor_copy(mod_sb[:, o0:o0 + ACH], mp[:, :ACH])
            nc.vector.tensor_scalar_add(mod_sb[:, SC_MSA:SC_MSA + H],
                                        mod_sb[:, SC_MSA:SC_MSA + H], 1.0)
            nc.vector.tensor_scalar_add(mod_sb[:, SC_MLP:SC_MLP + H],
                                        mod_sb[:, SC_MLP:SC_MLP + H], 1.0)

        def pool_lnbufs(pool):
            return 1 if pool.name.startswith("single:") or "ln2" in pool.name else 2

        def layer_norm_mod(pool, xt_f32, sh_off, sc_off, h_bf_out):
            stats = work.tile([P, 3, 6], F32, tag="lnstats")
            for ci in range(3):
                nc.vector.bn_stats(stats[:, ci], xt_f32[:, ci * 384:(ci + 1) * 384])
            aggr = work.tile([P, 2], F32, tag="lnaggr")
            nc.vector.bn_aggr(aggr[:], stats[:])
            std = work.tile([P, 1], F32, tag="lnstd")
            nc.scalar.activation(std[:], aggr[:, 1:2], AF.Sqrt, bias=eps_col[:])
            rstd = work.tile([P, 1], F32, tag="lnrstd")
            nc.vector.reciprocal(rstd[:], std[:])
            nmur = work.tile([P, 1], F32, tag="lnnmur")
            nc.vector.tensor_mul(nmur[:], aggr[:, 0:1], rstd[:])
            nc.vector.tensor_scalar_mul(nmur[:], nmur[:], -1.0)
            ln = pool.tile([P, H], F32, tag="lnout", bufs=pool_lnbufs(pool))
            nc.scalar.activation(ln[:], xt_f32, AF.Identity,
                                 bias=nmur[:], scale=rstd[:])
            tmp = pool.tile([P, H], F32, tag="lnmod", bufs=pool_lnbufs(pool))
            nc.vector.tensor_mul(tmp[:], ln[:], mod_sb[:, sc_off:sc_off + H])
            nc.vector.tensor_add(h_bf_out, tmp[:], mod_sb[:, sh_off:sh_off + H])

        def unpack_head(src_pk, h, width, scratch_pool, tag):
            """src_pk [128, NI, width] packed -> [72, width] bf16 scratch."""
            dst = scratch_pool.tile([HD, width], BF16, tag=tag, bufs=2)
            tiles = HEAD_TILES[h]
            for t2 in range(0, width, 512):
                t2w = min(512, width - t2)
                pq = psum(HD, t2w)
                for pi, (it, shift) in enumerate(tiles):
                    nc.tensor.matmul(
                        pq[:, :t2w],
                        sel[:, 128 + shift:200 + shift],
                        src_pk[:, it, t2:t2 + t2w],
                        start=(pi == 0), stop=(pi == len(tiles) - 1))
                nc.vector.tensor_copy(dst[:, t2:t2 + t2w], pq[:, :t2w])
            return dst

        def accum_packed(dst_pk, h, oh_bf, width):
            """dst_pk[:, it, :] (+)= selector(oh) for this head's tiles."""
            first = {it: (TILE_HEADS[it][0][0] == h) for it, _ in HEAD_TILES[h]}
            for (it, shift) in HEAD_TILES[h]:
                for t2 in range(0, width, 512):
                    t2w = min(512, width - t2)
                    rp = psum(P, t2w)
                    nc.tensor.matmul(
                        rp[:, :t2w],
                        sel[0:HD, 128 - shift:256 - shift],
                        oh_bf[:, t2:t2 + t2w],
                        start=True, stop=True)
                    if first[it]:
                        nc.vector.tensor_copy(dst_pk[:, it, t2:t2 + t2w],
                                              rp[:, :t2w])
                    else:
                        nc.vector.tensor_add(dst_pk[:, it, t2:t2 + t2w],
                                             dst_pk[:, it, t2:t2 + t2w],
                                             rp[:, :t2w])

        # =====================================================
        # Phases 1-3: LN1 -> QKV (packed) -> self-attention -> packed oPk
        # =====================================================
        with tc.tile_pool(name="opk", bufs=1) as opk_p:
            oPk = opk_p.tile([P, NI, N], BF16)

            with tc.tile_pool(name="qkvout", bufs=1) as qvo:
                qPk = qvo.tile([P, NI, N], BF16)
                kPk = qvo.tile([P, NI, N], BF16)
                v_aug = qvo.tile([P, NT, NH, 97], BF16)
                nc.vector.memset(v_aug[:], 1.0)

                with tc.tile_pool(name="qkvtmp", bufs=1) as qtp:
                    h1T = qtp.tile([P, NI, N], BF16)
                    with tc.tile_pool(name="ln1w", bufs=2) as lnw:
                        for t in range(NT):
                            xt = lnw.tile([P, H], F32, tag="xin")
                            nc.sync.dma_start(xt[:], x_d[t * P:(t + 1) * P, :])
                            h1 = lnw.tile([P, H], BF16, tag="h1")
                            layer_norm_mod(lnw, xt[:], SH_MSA, SC_MSA, h1[:])
                            for it in range(NI):
                                transpose_block(h1T[:, it, t * P:(t + 1) * P],
                                                h1[:, it * P:(it + 1) * P])

                    qkvb_col = qtp.tile([P, 2 * NI], F32)  # q|k bias columns
                    nc.sync.dma_start(
                        qkvb_col[:],
                        qkvb_d[0:2 * H].rearrange("(o p) -> p o", p=P))
                    vb_bf = qtp.tile([1, H], BF16)
                    nc.gpsimd.dma_start(vb_bf[:], qkvb_d[None, 2 * H:3 * H])

                    # ---- v (token-major) ----
                    with tc.tile_pool(name="vw", bufs=1) as vwp:
                        vT_w = vwp.tile([P, NI, H], BF16)
                        wT_load(vwp, vT_w, qkvw_d, 2 * H, H)
                        for t in range(NT):
                            for f0, fw, h0, nheads in VCH:
                                pv = psum(P, fw)
                                for it in range(NI):
                                    nc.tensor.matmul(
                                        pv[:, :fw],
                                        h1T[:, it, t * P:(t + 1) * P],
                                        vT_w[:, it, f0:f0 + fw],
                                        start=(it == 0), stop=False)
                                nc.tensor.matmul(
                                    pv[:, :fw], ones_row[:, :P],
                                    vb_bf[:, f0:f0 + fw],
                                    start=False, stop=True)
                                for hh in range(nheads):
                                    nc.vector.tensor_copy(
                                        v_aug[:, t, h0 + hh, 0:HD],
                                        pv[:, hh * HD:(hh + 1) * HD])

                    # ---- q and k, packed feature-major ----
                    for dst, wo0, bcol0 in ((qPk, 0, 0), (kPk, H, NI)):
                        with tc.tile_pool(name="qkw", bufs=1) as qkwp:
                            qw_T = qkwp.tile([P, NI, H], BF16)
                            wT_load(qkwp, qw_T, qkvw_d, wo0, H)
                            for ot in range(NI):
                                pq0 = psum()
                                pq1 = psum()
                                for it in range(NI):
                                    for t2, pq in ((0, pq0), (512, pq1)):
                                        nc.tensor.matmul(
                                            pq[:],
                                            qw_T[:, it, ot * P:(ot + 1) * P],
                                            h1T[:, it, t2:t2 + 512],
                                            start=(it == 0), stop=(it == NI - 1))
                                for t2, pq in ((0, pq0), (512, pq1)):
                                    nc.scalar.activation(
                                        dst[:, ot, t2:t2 + 512], pq[:],
                                        AF.Identity,
                                        bias=qkvb_col[:, bcol0 + ot:
                                                      bcol0 + ot + 1])

                # ---- self-attention ----
                with tc.tile_pool(name="attn", bufs=1) as atp:
                    for h in range(NH):
                        qh = unpack_head(qPk, h, N, atp, "qh")
                        kh = unpack_head(kPk, h, N, atp, "kh")
                        expST = atp.tile([P, NT, N], BF16, tag="expst", bufs=3)
                        for mt in range(NT):
                            for t2 in range(0, N, 512):
                                sp = psum()
                                nc.tensor.matmul(
                                    sp[:],
                                    kh[:, mt * P:(mt + 1) * P],
                                    qh[:, t2:t2 + 512],
                                    start=True, stop=True)
                                nc.scalar.activation(
                                    expST[:, mt, t2:t2 + 512], sp[:], AF.Exp,
                                    scale=ATT_SCALE)
                        po0 = psum(97)
                        po1 = psum(97)
                        for mt in range(NT):
                            for t2, po in ((0, po0), (512, po1)):
                                nc.tensor.matmul(
                                    po[:],
                                    v_aug[:, mt, h, :],
                                    expST[:, mt, t2:t2 + 512],
                                    start=(mt == 0), stop=(mt == NT - 1))
                        recip_bf = atp.tile([1, N], BF16, tag="recipbf", bufs=2)
                        oh = atp.tile([HD, N], BF16, tag="oh", bufs=2)
                        for t2, po in ((0, po0), (512, po1)):
                            recip = work.tile([1, 512], F32, tag="recip")
                            nc.vector.reciprocal(recip[:], po[96:97, :])
                            nc.vector.tensor_copy(
                                recip_bf[:, t2:t2 + 512], recip[:])
                            ohr = atp.tile([HD, 512], BF16, tag="ohr", bufs=2)
                            nc.vector.tensor_copy(ohr[:], po[0:HD, :])
                            rb = psum(HD)
                            nc.tensor.matmul(rb[:], ones_row[:, :HD],
                                             recip_bf[:, t2:t2 + 512],
                                             start=True, stop=True)
                            nc.vector.tensor_mul(oh[:, t2:t2 + 512],
                                                 ohr[:], rb[:])
                        accum_packed(oPk, h, oh, N)

            # ---- projection + gated residual -> x2 ----
            with tc.tile_pool(name="x2pool", bufs=1) as x2p:
                x2_sb = x2p.tile([P, NT, H], F32)
                with tc.tile_pool(name="projp", bufs=1) as pp:
                    projT = pp.tile([P, NI, H], BF16)
                    wT_load(pp, projT, projw_d, 0, H)
                    projb_bf = pp.tile([1, H], BF16)
                    nc.gpsimd.dma_start(projb_bf[:], projb_d[None, :])
                    for t in range(NT):
                        xin = pp.tile([P, H], F32, tag="xin2", bufs=2)
                        nc.sync.dma_start(xin[:], x_d[t * P:(t + 1) * P, :])
                        for f0 in range(0, H, 384):
                            pj = psum(P, 384)
                            for jt in range(NI):
                                nc.tensor.matmul(
                                    pj[:, :384],
                                    oPk[:, jt, t * P:(t + 1) * P],
                                    projT[:, jt, f0:f0 + 384],
                                    start=(jt == 0), stop=False)
                            nc.tensor.matmul(pj[:, :384], ones_row[:, :P],
                                             projb_bf[:, f0:f0 + 384],
                                             start=False, stop=True)
                            gp = pp.tile([P, 384], F32, tag="gp", bufs=2)
                            nc.vector.tensor_mul(
                                gp[:], pj[:, :384],
                                mod_sb[:, G_MSA + f0:G_MSA + f0 + 384])
                            nc.vector.tensor_add(
                                x2_sb[:, t, f0:f0 + 384],
                                xin[:, f0:f0 + 384], gp[:])

                # ---- cross-attention ----
                with tc.tile_pool(name="qxkeep", bufs=1) as qxk:
                    qxPk = qxk.tile([P, NI, N], BF16)
                    with tc.tile_pool(name="crossq", bufs=1) as cqp:
                        x2T = cqp.tile([P, NI, N], BF16)
                        for t in range(NT):
                            for it in range(NI):
                                transpose_block(
                                    x2T[:, it, t * P:(t + 1) * P],
                                    x2_sb[:, t, it * P:(it + 1) * P], f32=True)
                        tqT = cqp.tile([P, NI, H], BF16)
                        wT_load(cqp, tqT, tqw_d, 0, H)
                        for ot in range(NI):
                            pq0 = psum()
                            pq1 = psum()
                            for it in range(NI):
                                for t2, pq in ((0, pq0), (512, pq1)):
                                    nc.tensor.matmul(
                                        pq[:],
                                        tqT[:, it, ot * P:(ot + 1) * P],
                                        x2T[:, it, t2:t2 + 512],
                                        start=(it == 0), stop=(it == NI - 1))
                            for t2, pq in ((0, pq0), (512, pq1)):
                                nc.scalar.activation(
                                    qxPk[:, ot, t2:t2 + 512], pq[:],
                                    AF.Identity)

                    with tc.tile_pool(name="oxkeep", bufs=1) as oxk:
                        oxPk = oxk.tile([P, NI, N], BF16)
                        with tc.tile_pool(name="ctxp", bufs=1) as cxp:
                            c_bf = cxp.tile([CTXL, CTXD], BF16)
                            nc.gpsimd.dma_start(c_bf[:], c_d[:])
                            cT = cxp.tile([P, NIC, CTXL], BF16)
                            for it in range(NIC):
                                transpose_block(cT[:, it, :],
                                                c_bf[:, it * P:(it + 1) * P])
                            tkT = cxp.tile([P, NIC, H], BF16)
                            wT_load(cxp, tkT, tkw_d, 0, H, tag="wk")
                            kxPk = cxp.tile([P, NI, CTXL], BF16)
                            for ot in range(NI):
                                pk = psum(P, CTXL)
                                for it in range(NIC):
                                    nc.tensor.matmul(
                                        pk[:, :CTXL],
                                        tkT[:, it, ot * P:(ot + 1) * P],
                                        cT[:, it, :],
                                        start=(it == 0), stop=(it == NIC - 1))
                                nc.vector.tensor_copy(kxPk[:, ot, :],
                                                      pk[:, :CTXL])
                            kxT = cxp.tile([HD, NH, CTXL], BF16)
                            for h in range(NH):
                                pk = psum(HD, CTXL)
                                tiles = HEAD_TILES[h]
                                for pi, (it, shift) in enumerate(tiles):
                                    nc.tensor.matmul(
                                        pk[:, :CTXL],
                                        sel[:, 128 + shift:200 + shift],
                                        kxPk[:, it, :],
                                        start=(pi == 0),
                                        stop=(pi == len(tiles) - 1))
                                nc.vector.tensor_copy(kxT[:, h, :],
                                                      pk[:, :CTXL])

                            tvT = cxp.tile([P, NIC, H], BF16)
                            wT_load(cxp, tvT, tvw_d, 0, H, tag="wk")
                            vx_aug = cxp.tile([CTXL, NH, 97], BF16)
                            nc.vector.memset(vx_aug[:], 1.0)
                            for f0, fw, h0, nheads in VCH:
                                pv = psum(CTXL, fw)
                                for it in range(NIC):
                                    nc.tensor.matmul(
                                        pv[:, :fw], cT[:, it, :],
                                        tvT[:, it, f0:f0 + fw],
                                        start=(it == 0), stop=(it == NIC - 1))
                                for hh in range(nheads):
                                    nc.vector.tensor_copy(
                                        vx_aug[:, h0 + hh, 0:HD],
                                        pv[:, hh * HD:(hh + 1) * HD])

                            for h in range(NH):
                                qxh = unpack_head(qxPk, h, N, cxp, "qxh")
                                expS = cxp.tile([CTXL, N], BF16, tag="expsx",
                                                bufs=2)
                                for t2 in range(0, N, 512):
                                    spx = psum(CTXL)
                                    nc.tensor.matmul(
                                        spx[:],
                                        kxT[:, h, :],
                                        qxh[:, t2:t2 + 512],
                                        start=True, stop=True)
                                    nc.scalar.activation(
                                        expS[:, t2:t2 + 512], spx[:], AF.Exp,
                                        scale=ATT_SCALE)
                                recip_bf = cxp.tile([1, N], BF16,
                                                    tag="recipbf", bufs=2)
                                oxh = cxp.tile([HD, N], BF16, tag="oxh",
                                               bufs=2)
                                for t2 in range(0, N, 512):
                                    pox = psum(97)
                                    nc.tensor.matmul(
                                        pox[:],
                                        vx_aug[:, h, :],
                                        expS[:, t2:t2 + 512],
                                        start=True, stop=True)
                                    recip = work.tile([1, 512], F32,
                                                      tag="recip")
                                    nc.vector.reciprocal(recip[:],
                                                         pox[96:97, :])
                                    nc.vector.tensor_copy(
                                        recip_bf[:, t2:t2 + 512], recip[:])
                                    oxhr = cxp.tile([HD, 512], BF16,
                                                    tag="oxhr", bufs=2)
                                    nc.vector.tensor_copy(oxhr[:],
                                                          pox[0:HD, :])
                                    rb = psum(HD)
                                    nc.tensor.matmul(
                                        rb[:], ones_row[:, :HD],
                                        recip_bf[:, t2:t2 + 512],
                                        start=True, stop=True)
                                    nc.vector.tensor_mul(oxh[:, t2:t2 + 512],
                                                         oxhr[:], rb[:])
                                accum_packed(oxPk, h, oxh, N)

                        # ---- to_out + residual -> x3 (DRAM scratch) ----
                        with tc.tile_pool(name="towp", bufs=1) as top:
                            toT = top.tile([P, NI, H], BF16)
                            wT_load(top, toT, tow_d, 0, H)
                            tob_bf = top.tile([1, H], BF16)
                            nc.gpsimd.dma_start(tob_bf[:], tob_d[None, :])
                            for t in range(NT):
                                x3t = top.tile([P, H], F32, tag="x3t", bufs=2)
                                for f0 in range(0, H, 384):
                                    pj = psum(P, 384)
                                    for jt in range(NI):
                                        nc.tensor.matmul(
                                            pj[:, :384],
                                            oxPk[:, jt, t * P:(t + 1) * P],
                                            toT[:, jt, f0:f0 + 384],
                                            start=(jt == 0), stop=False)
                                    nc.tensor.matmul(
                                        pj[:, :384], ones_row[:, :P],
                                        tob_bf[:, f0:f0 + 384],
                                        start=False, stop=True)
                                    nc.vector.tensor_add(
                                        x3t[:, f0:f0 + 384],
                                        x2_sb[:, t, f0:f0 + 384], pj[:, :384])
                                nc.sync.dma_start(
                                    x3_d[t * P:(t + 1) * P, :], x3t[:])

        # =====================================================
        # Phase 6: MLP
        # =====================================================
        with tc.tile_pool(name="w2s", bufs=1) as w2s, \
             tc.tile_pool(name="mlp", bufs=1) as mp:
            b2_bf = w2s.tile([1, H], BF16, bufs=1)
            nc.gpsimd.dma_start(b2_bf[:], b2_d[None, :])

            def build_w2_chunk(f0):
                w2fc = w2s.tile([P, NJ, 384], BF16, tag="w2fc", name="w2fc")
                for s in range(3):
                    for jc in range(3):
                        nat = w2s.tile([P, 1536], BF16, tag="wnat1536",
                                       bufs=3, name="nat")
                        nc.gpsimd.dma_start(
                            nat[:],
                            w2_d[f0 + s * P:f0 + (s + 1) * P,
                                 jc * 1536:(jc + 1) * 1536])
                        nc.sync.dma_start(
                            out=w2fc[:, jc * 12:(jc + 1) * 12,
                                     s * P:(s + 1) * P],
                            in_=nat[:], transpose=True)
                return w2fc

            w2fc_tiles = {0: build_w2_chunk(0)}
            gT = mp.tile([P, NJ, N], BF16)
            with tc.tile_pool(name="mlp1", bufs=1) as m1p:
                h2T = m1p.tile([P, NI, N], BF16)
                with tc.tile_pool(name="ln2w", bufs=2) as lnw2:
                    for t in range(NT):
                        x3t = lnw2.tile([P, H], F32, tag="xin")
                        nc.sync.dma_start(x3t[:], x3_d[t * P:(t + 1) * P, :])
                        h2 = lnw2.tile([P, H], BF16, tag="h1")
                        layer_norm_mod(lnw2, x3t[:], SH_MLP, SC_MLP, h2[:])
                        for it in range(NI):
                            transpose_block(h2T[:, it, t * P:(t + 1) * P],
                                            h2[:, it * P:(it + 1) * P])

                b1_sb = m1p.tile([P, NJ], F32)
                nc.sync.dma_start(b1_sb[:], b1_d.rearrange("(o p) -> p o", p=P))
                for o in range(NJ):
                    w1blk = m1p.tile([P, NI, P], BF16, tag="w1blk", bufs=2)
                    wT_load(m1p, w1blk, w1_d, o * P, P)
                    pm0 = psum()
                    pm1 = psum()
                    for it in range(NI):
                        for t2, pm in ((0, pm0), (512, pm1)):
                            nc.tensor.matmul(
                                pm[:],
                                w1blk[:, it, :],
                                h2T[:, it, t2:t2 + 512],
                                start=(it == 0), stop=(it == NI - 1))
                    for t2, pm in ((0, pm0), (512, pm1)):
                        nc.scalar.activation(gT[:, o, t2:t2 + 512], pm[:],
                                             AF.Gelu_apprx_tanh,
                                             bias=b1_sb[:, o:o + 1])

            if True:
                for f0 in range(0, H, 384):
                    if f0 + 384 < H:
                        w2fc_tiles[f0 + 384] = build_w2_chunk(f0 + 384)
                    w2fc = w2fc_tiles.pop(f0)
                    for t in range(NT):
                        pj = psum(P, 384)
                        for j in range(NJ):
                            nc.tensor.matmul(
                                pj[:, :384],
                                gT[:, j, t * P:(t + 1) * P],
                                w2fc[:, j, :],
                                start=(j == 0), stop=False)
                        nc.tensor.matmul(pj[:, :384], ones_row[:, :P],
                                         b2_bf[:, f0:f0 + 384],
                                         start=False, stop=True)
                        x3c = w2s.tile([P, 384], F32, tag="x3c", bufs=2)
                        nc.sync.dma_start(
                            x3c[:], x3_d[t * P:(t + 1) * P, f0:f0 + 384])
                        gp = w2s.tile([P, 384], F32, tag="gp", bufs=2)
                        nc.vector.tensor_mul(
                            gp[:], pj[:, :384],
                            mod_sb[:, G_MLP + f0:G_MLP + f0 + 384])
                        oc = w2s.tile([P, 384], F32, tag="oc", bufs=2)
                        nc.vector.tensor_add(oc[:], x3c[:], gp[:])
                        nc.sync.dma_start(
                            out_d[t * P:(t + 1) * P, f0:f0 + 384], oc[:])

    if split_waits:
        _split_waits(nc)
    return nc


_BUILT = None


def _get_built():
    global _BUILT
    if _BUILT is None:
        nc = bass.Bass()
        build(nc)
        _BUILT = nc
    return _BUILT


def kernel(**inputs) -> np.ndarray:
    from concourse.bass_utils import run_bass_kernel_spmd

    nc = _get_built()
    keys = ["x", "c", "t_emb", "qkv_w", "qkv_b", "proj_w", "proj_b",
            "to_q_w", "to_k_w", "to_v_w", "to_out_w", "to_out_b",
            "mlp_w1", "mlp_b1", "mlp_w2", "mlp_b2", "ada_w", "ada_b"]
    in_maps = []
    for b in range(B):
        m = {}
        for k in keys:
            v = np.asarray(inputs[k], np.float32)
            if k in ("x", "c"):
                m[k] = np.ascontiguousarray(v[b])
            elif k == "t_emb":
                m[k] = np.ascontiguousarray(v[b:b + 1])
            else:
                m[k] = np.ascontiguousarray(v)
        in_maps.append(m)
    res = run_bass_kernel_spmd(nc, in_maps, list(range(B)))
    return np.stack([res.results[b]["out"] for b in range(B)], axis=0)



# revision 7
# speedup vs baseline: 1.1632x; 1.1632x over previous
"""DiT block (final) Trainium2 Bass kernel — data-parallel over batch.

Core b computes batch element b end-to-end (B=8 == n_cores), no collectives.
Host-side prep inside kernel(): all weights are pre-transposed to
[in_feat, out_feat] and pre-cast to bf16, then packed (with bf16 biases and
the per-core transposed context) into one bf16 buffer; x / t_emb / f32
biases pack into one f32 buffer.  This cuts PJRT dispatch to 3 buffers and
removes every on-device weight transpose / dtype cast.  GEMMs run on the PE
in bf16 (fp32 PSUM).  Heads (72 wide) are stored packed in 128-row tiles;
per-head operands are unpacked/repacked with shifted-diagonal selector
matmuls.  PSUM tiles are one bank drawn from an 8-slot pool so independent
accumulation chains pipeline.
"""

import contextlib

import numpy as np

import concourse.bass as bass
import concourse.tile as tile
import concourse.mybir as mybir
from concourse.masks import make_identity
from concourse.vector_clock import ScopedClock

F32 = mybir.dt.float32
BF16 = mybir.dt.bfloat16
AF = mybir.ActivationFunctionType

B = 8
N = 1024
H = 1152
NH = 16
HD = 72
CTXL = 77
CTXD = 768
MLP = 4608
EPS = 1e-6
P = 128
NT = N // P       # 8
NI = H // P       # 9
NIC = CTXD // P   # 6
NJ = MLP // P     # 36
ATT_SCALE = HD ** -0.5

SH_MSA, SC_MSA, G_MSA = 0, H, 2 * H
SH_MLP, SC_MLP, G_MLP = 3 * H, 4 * H, 5 * H

# whole-head output chunks for v projections: (f0, width, first head, n heads)
VCH = [(0, 360, 0, 5), (360, 360, 5, 5), (720, 360, 10, 5), (1080, 72, 15, 1)]

# ---- packed_f32 layout (element offsets) ----
XO = 0
TEO = XO + N * H                 # 1179648
QKVBO = TEO + H                  # 1180800
B1O = QKVBO + 3 * H              # 1184256
F32TOT = B1O + MLP               # 1188864

# ---- packed_bf16 layout: W.T = [in, out] row-major ----
QKVW = 0                          # [H, 3H]
PROJW = QKVW + H * 3 * H          # [H, H]
TQW = PROJW + H * H
TKW = TQW + H * H                 # [CTXD, H]
TVW = TKW + CTXD * H
TOW = TVW + CTXD * H              # [H, H]
W1 = TOW + H * H                  # [H, MLP]
W2 = W1 + H * MLP                 # [MLP, H]
ADAW = W2 + MLP * H               # [H, 6H]
VBB = ADAW + H * 6 * H            # qkv_b[2H:3H] bf16 [H]
PROJBB = VBB + H
TOBB = PROJBB + H
B2B = TOBB + H
ADABB = B2B + H                   # [6H]
CTB = ADABB + 6 * H               # c.T bf16 [CTXD, CTXL]
BF16TOT = CTB + CTXD * CTXL


class SplitDrainTileContext(tile.TileContext):
    """This walrus build rejects a Drain carrying >1 sem wait; split the
    end-of-kernel drain into one drain per wait."""

    def _drain_and_barrier(self, tick_clock, wait_clock):
        nc = self.nc
        drain_bi = nc.sync.drain()
        wait_clock.add_sem_waits(
            drain_bi.ins, ScopedClock({None: tick_clock.global_clock})
        )
        si = drain_bi.ins.sync_info
        waits = list(si.on_wait or [])
        if len(waits) > 1:
            del si.on_wait[1:]
            for w in waits[1:]:
                d2 = nc.sync.drain()
                d2.ins.sync_info = mybir.SyncInfo(on_wait=[w], on_update=[])
        nc.all_engine_barrier()
        popped = nc._tile_sem_poison_stack.pop()
        assert popped is self._sem_poison
        nc.clear_and_free_semaphores(list(self.sems.allocated().values()))
        nc.all_engine_barrier()


def _split_waits(nc, limit=1):
    """This walrus build supports very few sync-wait slots per instruction.
    Move extra waits onto same-engine NOPs inserted just before each
    instruction (in-order engines make this semantics-preserving)."""
    n_new = 0
    for f in nc.m.functions:
        for blk in f.blocks:
            insts = list(blk.instructions)
            out = []
            changed = False
            for inst in insts:
                si = getattr(inst, "sync_info", None)
                eng = getattr(inst, "engine", None)
                if (si and si.on_wait and len(si.on_wait) > limit
                        and eng is not None
                        and eng != mybir.EngineType.Unassigned):
                    waits = list(si.on_wait)
                    del si.on_wait[:]
                    si.on_wait.extend(waits[-limit:])
                    for w in waits[:-limit]:
                        n_new += 1
                        nop = mybir.InstNoOp(
                            name=f"wsplit-{n_new}-{inst.name}", ins=[], outs=[])
                        nop.engine = eng
                        nop.sync_info = mybir.SyncInfo(on_wait=[w], on_update=[])
                        out.append(nop)
                    changed = True
                out.append(inst)
            if changed:
                blk.instructions[:] = out
    return n_new


def _head_tiles():
    """Head h spans packed rows [72h, 72h+72): list of (it, shift) where
    shift = 72h - 128it; packed tile `it` row r holds head dim r - shift."""
    out = []
    for h in range(NH):
        lo, hi = HD * h, HD * (h + 1)
        out.append([(it, HD * h - P * it)
                    for it in range(lo // P, (hi - 1) // P + 1)])
    return out


HEAD_TILES = _head_tiles()
TILE_HEADS = [[] for _ in range(NI)]
for _h, _tiles in enumerate(HEAD_TILES):
    for _it, _s in _tiles:
        TILE_HEADS[_it].append((_h, _s))


def build(nc: bass.Bass, split_waits=True):
    pf_d = nc.dram_tensor("packed_f32", [F32TOT], F32, kind="ExternalInput")
    pb_d = nc.dram_tensor("packed_bf16", [BF16TOT], BF16, kind="ExternalInput")
    out_d = nc.dram_tensor("out", [N, H], F32, kind="ExternalOutput")
    x3_d = nc.dram_tensor("x3_scratch", [N, H], F32)

    def pfv(off, n, m):
        return pf_d[off:off + n * m].rearrange("(a b) -> a b", a=n)

    def pbv(off, n, m):
        return pb_d[off:off + n * m].rearrange("(a b) -> a b", a=n)

    def wt_rows(base, out_len, it, o0, o_w):
        """[128, o_w] view: W.T rows [128it, 128it+128), cols [o0, o0+o_w)."""
        return pbv(base + it * P * out_len, P, out_len)[:, o0:o0 + o_w]

    def x_rows(t):
        return pfv(XO + t * P * H, P, H)

    with SplitDrainTileContext(nc) as tc, contextlib.ExitStack() as ctx:
        EC = ctx.enter_context
        const = EC(tc.tile_pool(name="const", bufs=1))
        persist = EC(tc.tile_pool(name="persist", bufs=1))
        work = EC(tc.tile_pool(name="work", bufs=3))              # tiny scratch
        psF = EC(tc.tile_pool(name="psF", bufs=8, space="PSUM"))  # 1-bank slots

        def psum(parts=P, width=512):
            return psF.tile([parts, width], F32, tag="f", name="ps")

        identb = const.tile([P, P], BF16)
        make_identity(nc, identb[:])
        identf = const.tile([P, P], F32)
        make_identity(nc, identf[:])
        ones_row = const.tile([1, 512], BF16)
        nc.vector.memset(ones_row[:], 1.0)
        eps_col = const.tile([P, 1], F32)
        nc.vector.memset(eps_col[:], EPS)
        # shifted diagonal: sel[d, c] = 1 iff c == d + 128
        sel = const.tile([P, 328], BF16)
        nc.gpsimd.memset(sel[:], 0.0)
        nc.gpsimd.affine_select(
            out=sel[:], in_=sel[:], compare_op=mybir.AluOpType.not_equal,
            fill=1.0, base=128, pattern=[[-1, 328]], channel_multiplier=1)

        mod_sb = persist.tile([P, 6 * H], BF16)

        tb_state = [0]

        def transpose_block(dst_bf, src, f32=False):
            p = src.shape[0]
            ident = identf if f32 else identb
            pt = psF.tile([P, P + P], F32 if f32 else BF16, tag="f", name="pt")
            nc.tensor.transpose(pt[:, :p], src, ident[0:p, 0:p])
            tb_state[0] ^= 1
            if tb_state[0]:
                nc.vector.tensor_copy(dst_bf, pt[:, :p])
            else:
                nc.scalar.activation(dst_bf, pt[:, :p], AF.Identity)

        # =====================================================
        # Phase 0: adaLN modulation table (pre-broadcast over partitions)
        # =====================================================
        temb_sb = work.tile([1, H], F32, tag="temb", bufs=1)
        nc.sync.dma_start(temb_sb[:], pfv(TEO, 1, H))
        s_bf = work.tile([1, H], BF16, tag="sbf", bufs=1)
        nc.scalar.activation(s_bf[:], temb_sb[:], AF.Silu)

        with tc.tile_pool(name="p0", bufs=1) as p0, \
             tc.tile_pool(name="p0w", bufs=2) as p0w:
            sbc_sb = p0.tile([P, H], BF16)
            for f0 in range(0, H, 512):
                w = min(512, H - f0)
                sp = psum()
                nc.tensor.matmul(sp[:, :w], ones_row[:, :P],
                                 s_bf[:, f0:f0 + w], start=True, stop=True)
                nc.vector.tensor_copy(sbc_sb[:, f0:f0 + w], sp[:, :w])
            s_rep = p0.tile([P, NI, P], BF16)
            for it in range(NI):
                transpose_block(s_rep[:, it, :], sbc_sb[:, it * P:(it + 1) * P])

            adab_bf = p0.tile([1, 6 * H], BF16)
            nc.sync.dma_start(adab_bf[:], pb_d[None, ADABB:ADABB + 6 * H])
            ACH = 384
            for o0 in range(0, 6 * H, ACH):
                adat = p0w.tile([P, NI, ACH], BF16, tag="adat")
                for it in range(NI):
                    nc.sync.dma_start(adat[:, it, :],
                                      wt_rows(ADAW, 6 * H, it, o0, ACH))
                mp = psum()
                for it in range(NI):
                    nc.tensor.matmul(mp[:, :ACH], s_rep[:, it, :],
                                     adat[:, it, :], start=(it == 0), stop=False)
                nc.tensor.matmul(mp[:, :ACH], ones_row[:, :P],
                                 adab_bf[:, o0:o0 + ACH], start=False, stop=True)
                nc.vector.tensor_copy(mod_sb[:, o0:o0 + ACH], mp[:, :ACH])
            nc.vector.tensor_scalar_add(mod_sb[:, SC_MSA:SC_MSA + H],
                                        mod_sb[:, SC_MSA:SC_MSA + H], 1.0)
            nc.vector.tensor_scalar_add(mod_sb[:, SC_MLP:SC_MLP + H],
                                        mod_sb[:, SC_MLP:SC_MLP + H], 1.0)

        def pool_lnbufs(pool):
            return 1 if pool.name.startswith("single:") or "ln2" in pool.name else 2

        def layer_norm_mod(pool, xt_f32, sh_off, sc_off, h_bf_out):
            stats = work.tile([P, 3, 6], F32, tag="lnstats")
            for ci in range(3):
                nc.vector.bn_stats(stats[:, ci], xt_f32[:, ci * 384:(ci + 1) * 384])
            aggr = work.tile([P, 2], F32, tag="lnaggr")
            nc.vector.bn_aggr(aggr[:], stats[:])
            std = work.tile([P, 1], F32, tag="lnstd")
            nc.scalar.activation(std[:], aggr[:, 1:2], AF.Sqrt, bias=eps_col[:])
            rstd = work.tile([P, 1], F32, tag="lnrstd")
            nc.vector.reciprocal(rstd[:], std[:])
            nmur = work.tile([P, 1], F32, tag="lnnmur")
            nc.vector.tensor_mul(nmur[:], aggr[:, 0:1], rstd[:])
            nc.vector.tensor_scalar_mul(nmur[:], nmur[:], -1.0)
            ln = pool.tile([P, H], F32, tag="lnout", bufs=pool_lnbufs(pool))
            nc.scalar.activation(ln[:], xt_f32, AF.Identity,
                                 bias=nmur[:], scale=rstd[:])
            nc.vector.tensor_mul(ln[:], ln[:], mod_sb[:, sc_off:sc_off + H])
            nc.vector.tensor_add(h_bf_out, ln[:], mod_sb[:, sh_off:sh_off + H])

        def unpack_head(src_pk, h, width, scratch_pool, tag):
            """src_pk [128, NI, width] packed -> [72, width] bf16 scratch."""
            dst = scratch_pool.tile([HD, width], BF16, tag=tag, bufs=2)
            tiles = HEAD_TILES[h]
            for t2 in range(0, width, 512):
                t2w = min(512, width - t2)
                pq = psum(HD, t2w)
                for pi, (it, shift) in enumerate(tiles):
                    nc.tensor.matmul(
                        pq[:, :t2w],
                        sel[:, 128 + shift:200 + shift],
                        src_pk[:, it, t2:t2 + t2w],
                        start=(pi == 0), stop=(pi == len(tiles) - 1))
                nc.vector.tensor_copy(dst[:, t2:t2 + t2w], pq[:, :t2w])
            return dst

        def accum_packed(dst_pk, h, oh_bf, width):
            """dst_pk[:, it, :] (+)= selector(oh) for this head's tiles."""
            first = {it: (TILE_HEADS[it][0][0] == h) for it, _ in HEAD_TILES[h]}
            for (it, shift) in HEAD_TILES[h]:
                for t2 in range(0, width, 512):
                    t2w = min(512, width - t2)
                    rp = psum(P, t2w)
                    nc.tensor.matmul(
                        rp[:, :t2w],
                        sel[0:HD, 128 - shift:256 - shift],
                        oh_bf[:, t2:t2 + t2w],
                        start=True, stop=True)
                    if first[it]:
                        nc.vector.tensor_copy(dst_pk[:, it, t2:t2 + t2w],
                                              rp[:, :t2w])
                    else:
                        nc.vector.tensor_add(dst_pk[:, it, t2:t2 + t2w],
                                             dst_pk[:, it, t2:t2 + t2w],
                                             rp[:, :t2w])

        # =====================================================
        # Phases 1-3: LN1 -> QKV (packed) -> self-attention -> packed oPk
        # =====================================================
        with tc.tile_pool(name="opk", bufs=1) as opk_p:
            oPk = opk_p.tile([P, NI, N], BF16)

            with tc.tile_pool(name="qkvout", bufs=1) as qvo:
                qPk = qvo.tile([P, NI, N], BF16)
                kPk = qvo.tile([P, NI, N], BF16)
                v_aug = qvo.tile([P, NT, NH, 97], BF16)
                nc.vector.memset(v_aug[:], 1.0)

                with tc.tile_pool(name="qkvtmp", bufs=1) as qtp:
                    h1T = qtp.tile([P, NI, N], BF16)
                    with tc.tile_pool(name="ln1w", bufs=2) as lnw:
                        for t in range(NT):
                            xt = lnw.tile([P, H], F32, tag="xin")
                            nc.sync.dma_start(xt[:], x_rows(t))
                            h1 = lnw.tile([P, H], BF16, tag="h1")
                            layer_norm_mod(lnw, xt[:], SH_MSA, SC_MSA, h1[:])
                            for it in range(NI):
                                transpose_block(h1T[:, it, t * P:(t + 1) * P],
                                                h1[:, it * P:(it + 1) * P])

                    qkvb_col = qtp.tile([P, 2 * NI], F32)  # q|k bias columns
                    nc.sync.dma_start(
                        qkvb_col[:],
                        pf_d[QKVBO:QKVBO + 2 * H].rearrange("(o p) -> p o", p=P))
                    vb_bf = qtp.tile([1, H], BF16)
                    nc.sync.dma_start(vb_bf[:], pb_d[None, VBB:VBB + H])

                    # ---- v (token-major) ----
                    with tc.tile_pool(name="vw", bufs=1) as vwp:
                        vT_w = vwp.tile([P, NI, H], BF16)
                        for it in range(NI):
                            nc.sync.dma_start(
                                vT_w[:, it, :],
                                wt_rows(QKVW, 3 * H, it, 2 * H, H))
                        for t in range(NT):
                            for f0, fw, h0, nheads in VCH:
                                pv = psum(P, fw)
                                for it in range(NI):
                                    nc.tensor.matmul(
                                        pv[:, :fw],
                                        h1T[:, it, t * P:(t + 1) * P],
                                        vT_w[:, it, f0:f0 + fw],
                                        start=(it == 0), stop=False)
                                nc.tensor.matmul(
                                    pv[:, :fw], ones_row[:, :P],
                                    vb_bf[:, f0:f0 + fw],
                                    start=False, stop=True)
                                for hh in range(nheads):
                                    nc.vector.tensor_copy(
                                        v_aug[:, t, h0 + hh, 0:HD],
                                        pv[:, hh * HD:(hh + 1) * HD])

                    # ---- q and k, packed feature-major ----
                    for dst, wo0, bcol0 in ((qPk, 0, 0), (kPk, H, NI)):
                        with tc.tile_pool(name="qkw", bufs=1) as qkwp:
                            qw_T = qkwp.tile([P, NI, H], BF16)
                            for it in range(NI):
                                nc.sync.dma_start(
                                    qw_T[:, it, :],
                                    wt_rows(QKVW, 3 * H, it, wo0, H))
                            for ot in range(NI):
                                pq0 = psum()
                                pq1 = psum()
                                for it in range(NI):
                                    for t2, pq in ((0, pq0), (512, pq1)):
                                        nc.tensor.matmul(
                                            pq[:],
                                            qw_T[:, it, ot * P:(ot + 1) * P],
                                            h1T[:, it, t2:t2 + 512],
                                            start=(it == 0), stop=(it == NI - 1))
                                for t2, pq in ((0, pq0), (512, pq1)):
                                    nc.scalar.activation(
                                        dst[:, ot, t2:t2 + 512], pq[:],
                                        AF.Identity,
                                        bias=qkvb_col[:, bcol0 + ot:
                                                      bcol0 + ot + 1])

                # ---- self-attention ----
                with tc.tile_pool(name="attn", bufs=1) as atp:
                    for h in range(NH):
                        qh = unpack_head(qPk, h, N, atp, "qh")
                        kh = unpack_head(kPk, h, N, atp, "kh")
                        expST = atp.tile([P, NT, N], BF16, tag="expst", bufs=3)
                        for mt in range(NT):
                            for t2 in range(0, N, 512):
                                sp = psum()
                                nc.tensor.matmul(
                                    sp[:],
                                    kh[:, mt * P:(mt + 1) * P],
                                    qh[:, t2:t2 + 512],
                                    start=True, stop=True)
                                nc.scalar.activation(
                                    expST[:, mt, t2:t2 + 512], sp[:], AF.Exp,
                                    scale=ATT_SCALE)
                        po0 = psum(97)
                        po1 = psum(97)
                        for mt in range(NT):
                            for t2, po in ((0, po0), (512, po1)):
                                nc.tensor.matmul(
                                    po[:],
                                    v_aug[:, mt, h, :],
                                    expST[:, mt, t2:t2 + 512],
                                    start=(mt == 0), stop=(mt == NT - 1))
                        recip_bf = atp.tile([1, N], BF16, tag="recipbf", bufs=2)
                        oh = atp.tile([HD, N], BF16, tag="oh", bufs=2)
                        for t2, po in ((0, po0), (512, po1)):
                            recip = work.tile([1, 512], F32, tag="recip")
                            nc.vector.reciprocal(recip[:], po[96:97, :])
                            nc.vector.tensor_copy(
                                recip_bf[:, t2:t2 + 512], recip[:])
                            ohr = atp.tile([HD, 512], BF16, tag="ohr", bufs=2)
                            nc.vector.tensor_copy(ohr[:], po[0:HD, :])
                            rb = psum(HD)
                            nc.tensor.matmul(rb[:], ones_row[:, :HD],
                                             recip_bf[:, t2:t2 + 512],
                                             start=True, stop=True)
                            nc.vector.tensor_mul(oh[:, t2:t2 + 512],
                                                 ohr[:], rb[:])
                        accum_packed(oPk, h, oh, N)

            # ---- projection + gated residual -> x2 ----
            with tc.tile_pool(name="x2pool", bufs=1) as x2p:
                x2_sb = x2p.tile([P, NT, H], F32)
                with tc.tile_pool(name="projp", bufs=1) as pp:
                    projT = pp.tile([P, NI, H], BF16)
                    for it in range(NI):
                        nc.sync.dma_start(projT[:, it, :],
                                          wt_rows(PROJW, H, it, 0, H))
                    projb_bf = pp.tile([1, H], BF16)
                    nc.sync.dma_start(projb_bf[:], pb_d[None, PROJBB:PROJBB + H])
                    for t in range(NT):
                        xin = pp.tile([P, H], F32, tag="xin2", bufs=2)
                        nc.sync.dma_start(xin[:], x_rows(t))
                        for f0 in range(0, H, 384):
                            pj = psum(P, 384)
                            for jt in range(NI):
                                nc.tensor.matmul(
                                    pj[:, :384],
                                    oPk[:, jt, t * P:(t + 1) * P],
                                    projT[:, jt, f0:f0 + 384],
                                    start=(jt == 0), stop=False)
                            nc.tensor.matmul(pj[:, :384], ones_row[:, :P],
                                             projb_bf[:, f0:f0 + 384],
                                             start=False, stop=True)
                            gp = pp.tile([P, 384], F32, tag="gp", bufs=2)
                            nc.vector.tensor_mul(
                                gp[:], pj[:, :384],
                                mod_sb[:, G_MSA + f0:G_MSA + f0 + 384])
                            nc.vector.tensor_add(
                                x2_sb[:, t, f0:f0 + 384],
                                xin[:, f0:f0 + 384], gp[:])

                # ---- cross-attention ----
                with tc.tile_pool(name="qxkeep", bufs=1) as qxk:
                    qxPk = qxk.tile([P, NI, N], BF16)
                    with tc.tile_pool(name="crossq", bufs=1) as cqp:
                        x2T = cqp.tile([P, NI, N], BF16)
                        for t in range(NT):
                            for it in range(NI):
                                transpose_block(
                                    x2T[:, it, t * P:(t + 1) * P],
                                    x2_sb[:, t, it * P:(it + 1) * P], f32=True)
                        tqT = cqp.tile([P, NI, H], BF16)
                        for it in range(NI):
                            nc.sync.dma_start(tqT[:, it, :],
                                              wt_rows(TQW, H, it, 0, H))
                        for ot in range(NI):
                            pq0 = psum()
                            pq1 = psum()
                            for it in range(NI):
                                for t2, pq in ((0, pq0), (512, pq1)):
                                    nc.tensor.matmul(
                                        pq[:],
                                        tqT[:, it, ot * P:(ot + 1) * P],
                                        x2T[:, it, t2:t2 + 512],
                                        start=(it == 0), stop=(it == NI - 1))
                            for t2, pq in ((0, pq0), (512, pq1)):
                                nc.scalar.activation(
                                    qxPk[:, ot, t2:t2 + 512], pq[:],
                                    AF.Identity)

                    with tc.tile_pool(name="oxkeep", bufs=1) as oxk:
                        oxPk = oxk.tile([P, NI, N], BF16)
                        with tc.tile_pool(name="ctxp", bufs=1) as cxp:
                            cT = cxp.tile([P, NIC, CTXL], BF16)
                            for it in range(NIC):
                                nc.sync.dma_start(
                                    cT[:, it, :],
                                    pbv(CTB + it * P * CTXL, P, CTXL))
                            tkT = cxp.tile([P, NIC, H], BF16)
                            for it in range(NIC):
                                nc.sync.dma_start(tkT[:, it, :],
                                                  wt_rows(TKW, H, it, 0, H))
                            kxPk = cxp.tile([P, NI, CTXL], BF16)
                            for ot in range(NI):
                                pk = psum(P, CTXL)
                                for it in range(NIC):
                                    nc.tensor.matmul(
                                        pk[:, :CTXL],
                                        tkT[:, it, ot * P:(ot + 1) * P],
                                        cT[:, it, :],
                                        start=(it == 0), stop=(it == NIC - 1))
                                nc.vector.tensor_copy(kxPk[:, ot, :],
                                                      pk[:, :CTXL])
                            kxT = cxp.tile([HD, NH, CTXL], BF16)
                            for h in range(NH):
                                pk = psum(HD, CTXL)
                                tiles = HEAD_TILES[h]
                                for pi, (it, shift) in enumerate(tiles):
                                    nc.tensor.matmul(
                                        pk[:, :CTXL],
                                        sel[:, 128 + shift:200 + shift],
                                        kxPk[:, it, :],
                                        start=(pi == 0),
                                        stop=(pi == len(tiles) - 1))
                                nc.vector.tensor_copy(kxT[:, h, :],
                                                      pk[:, :CTXL])

                            tvT = cxp.tile([P, NIC, H], BF16)
                            for it in range(NIC):
                                nc.sync.dma_start(tvT[:, it, :],
                                                  wt_rows(TVW, H, it, 0, H))
                            vx_aug = cxp.tile([CTXL, NH, 97], BF16)
                            nc.vector.memset(vx_aug[:], 1.0)
                            for f0, fw, h0, nheads in VCH:
                                pv = psum(CTXL, fw)
                                for it in range(NIC):
                                    nc.tensor.matmul(
                                        pv[:, :fw], cT[:, it, :],
                                        tvT[:, it, f0:f0 + fw],
                                        start=(it == 0), stop=(it == NIC - 1))
                                for hh in range(nheads):
                                    nc.vector.tensor_copy(
                                        vx_aug[:, h0 + hh, 0:HD],
                                        pv[:, hh * HD:(hh + 1) * HD])

                            for h in range(NH):
                                qxh = unpack_head(qxPk, h, N, cxp, "qxh")
                                expS = cxp.tile([CTXL, N], BF16, tag="expsx",
                                                bufs=2)
                                for t2 in range(0, N, 512):
                                    spx = psum(CTXL)
                                    nc.tensor.matmul(
                                        spx[:],
                                        kxT[:, h, :],
                                        qxh[:, t2:t2 + 512],
                                        start=True, stop=True)
                                    nc.scalar.activation(
                                        expS[:, t2:t2 + 512], spx[:], AF.Exp,
                                        scale=ATT_SCALE)
                                recip_bf = cxp.tile([1, N], BF16,
                                                    tag="recipbf", bufs=2)
                                oxh = cxp.tile([HD, N], BF16, tag="oxh",
                                               bufs=2)
                                for t2 in range(0, N, 512):
                                    pox = psum(97)
                                    nc.tensor.matmul(
                                        pox[:],
                                        vx_aug[:, h, :],
                                        expS[:, t2:t2 + 512],
                                        start=True, stop=True)
                                    recip = work.tile([1, 512], F32,
                                                      tag="recip")
                                    nc.vector.reciprocal(recip[:],
                                                         pox[96:97, :])
                                    nc.vector.tensor_copy(
                                        recip_bf[:, t2:t2 + 512], recip[:])
                                    oxhr = cxp.tile([HD, 512], BF16,
                                                    tag="oxhr", bufs=2)
                                    nc.vector.tensor_copy(oxhr[:],
                                                          pox[0:HD, :])
                                    rb = psum(HD)
                                    nc.tensor.matmul(
                                        rb[:], ones_row[:, :HD],
                                        recip_bf[:, t2:t2 + 512],
                                        start=True, stop=True)
                                    nc.vector.tensor_mul(oxh[:, t2:t2 + 512],
                                                         oxhr[:], rb[:])
                                accum_packed(oxPk, h, oxh, N)

                        # ---- to_out + residual -> x3 (DRAM scratch) ----
                        with tc.tile_pool(name="towp", bufs=1) as top:
                            toT = top.tile([P, NI, H], BF16)
                            for it in range(NI):
                                nc.sync.dma_start(toT[:, it, :],
                                                  wt_rows(TOW, H, it, 0, H))
                            tob_bf = top.tile([1, H], BF16)
                            nc.sync.dma_start(tob_bf[:],
                                              pb_d[None, TOBB:TOBB + H])
                            for t in range(NT):
                                x3t = top.tile([P, H], F32, tag="x3t", bufs=2)
                                for f0 in range(0, H, 384):
                                    pj = psum(P, 384)
                                    for jt in range(NI):
                                        nc.tensor.matmul(
                                            pj[:, :384],
                                            oxPk[:, jt, t * P:(t + 1) * P],
                                            toT[:, jt, f0:f0 + 384],
                                            start=(jt == 0), stop=False)
                                    nc.tensor.matmul(
                                        pj[:, :384], ones_row[:, :P],
                                        tob_bf[:, f0:f0 + 384],
                                        start=False, stop=True)
                                    nc.vector.tensor_add(
                                        x3t[:, f0:f0 + 384],
                                        x2_sb[:, t, f0:f0 + 384], pj[:, :384])
                                nc.sync.dma_start(
                                    x3_d[t * P:(t + 1) * P, :], x3t[:])

        # =====================================================
        # Phase 6: MLP
        # =====================================================
        with tc.tile_pool(name="w2s", bufs=1) as w2s, \
             tc.tile_pool(name="mlp", bufs=1) as mp:
            b2_bf = w2s.tile([1, H], BF16, bufs=1)
            nc.sync.dma_start(b2_bf[:], pb_d[None, B2B:B2B + H])

            def build_w2_chunk(f0):
                w2fc = w2s.tile([P, NJ, 384], BF16, tag="w2fc", name="w2fc")
                for jt in range(NJ):
                    nc.sync.dma_start(w2fc[:, jt, :],
                                      wt_rows(W2, H, jt, f0, 384))
                return w2fc

            w2fc_tiles = {0: build_w2_chunk(0)}
            gT = mp.tile([P, NJ, N], BF16)
            with tc.tile_pool(name="mlp1", bufs=1) as m1p:
                h2T = m1p.tile([P, NI, N], BF16)
                with tc.tile_pool(name="ln2w", bufs=2) as lnw2:
                    for t in range(NT):
                        x3t = lnw2.tile([P, H], F32, tag="xin")
                        nc.sync.dma_start(x3t[:], x3_d[t * P:(t + 1) * P, :])
                        h2 = lnw2.tile([P, H], BF16, tag="h1")
                        layer_norm_mod(lnw2, x3t[:], SH_MLP, SC_MLP, h2[:])
                        for it in range(NI):
                            transpose_block(h2T[:, it, t * P:(t + 1) * P],
                                            h2[:, it * P:(it + 1) * P])

                b1_sb = m1p.tile([P, NJ], F32)
                nc.sync.dma_start(
                    b1_sb[:],
                    pf_d[B1O:B1O + MLP].rearrange("(o p) -> p o", p=P))
                OW = 768
                for ow in range(0, MLP, OW):
                    w1win = m1p.tile([P, NI, OW], BF16, tag="w1win", bufs=2)
                    for it in range(NI):
                        nc.sync.dma_start(w1win[:, it, :],
                                          wt_rows(W1, MLP, it, ow, OW))
                    for oi in range(OW // P):
                        o = ow // P + oi
                        pm0 = psum()
                        pm1 = psum()
                        for it in range(NI):
                            for t2, pm in ((0, pm0), (512, pm1)):
                                nc.tensor.matmul(
                                    pm[:],
                                    w1win[:, it, oi * P:(oi + 1) * P],
                                    h2T[:, it, t2:t2 + 512],
                                    start=(it == 0), stop=(it == NI - 1))
                        for t2, pm in ((0, pm0), (512, pm1)):
                            nc.scalar.activation(gT[:, o, t2:t2 + 512], pm[:],
                                                 AF.Gelu_apprx_tanh,
                                                 bias=b1_sb[:, o:o + 1])

            for f0 in range(0, H, 384):
                if f0 + 384 < H:
                    w2fc_tiles[f0 + 384] = build_w2_chunk(f0 + 384)
                w2fc = w2fc_tiles.pop(f0)
                for t in range(NT):
                    pj = psum(P, 384)
                    for j in range(NJ):
                        nc.tensor.matmul(
                            pj[:, :384],
                            gT[:, j, t * P:(t + 1) * P],
                            w2fc[:, j, :],
                            start=(j == 0), stop=False)
                    nc.tensor.matmul(pj[:, :384], ones_row[:, :P],
                                     b2_bf[:, f0:f0 + 384],
                                     start=False, stop=True)
                    x3c = w2s.tile([P, 384], F32, tag="x3c", bufs=2)
                    nc.sync.dma_start(
                        x3c[:], x3_d[t * P:(t + 1) * P, f0:f0 + 384])
                    gp = w2s.tile([P, 384], F32, tag="gp", bufs=2)
                    nc.vector.tensor_mul(
                        gp[:], pj[:, :384],
                        mod_sb[:, G_MLP + f0:G_MLP + f0 + 384])
                    oc = w2s.tile([P, 384], F32, tag="oc", bufs=2)
                    nc.vector.tensor_add(oc[:], x3c[:], gp[:])
                    nc.sync.dma_start(
                        out_d[t * P:(t + 1) * P, f0:f0 + 384], oc[:])

    if split_waits:
        _split_waits(nc)
    return nc


def make_in_maps(inputs):
    """Host-side packing: per-core {packed_f32, packed_bf16} buffers."""
    import ml_dtypes
    BD = ml_dtypes.bfloat16

    def f32(k):
        return np.asarray(inputs[k], np.float32)

    # shared bf16 weight image (identical across cores)
    wparts = [
        np.ascontiguousarray(f32("qkv_w").T).astype(BD).ravel(),
        np.ascontiguousarray(f32("proj_w").T).astype(BD).ravel(),
        np.ascontiguousarray(f32("to_q_w").T).astype(BD).ravel(),
        np.ascontiguousarray(f32("to_k_w").T).astype(BD).ravel(),
        np.ascontiguousarray(f32("to_v_w").T).astype(BD).ravel(),
        np.ascontiguousarray(f32("to_out_w").T).astype(BD).ravel(),
        np.ascontiguousarray(f32("mlp_w1").T).astype(BD).ravel(),
        np.ascontiguousarray(f32("mlp_w2").T).astype(BD).ravel(),
        np.ascontiguousarray(f32("ada_w").T).astype(BD).ravel(),
        f32("qkv_b")[2 * H:3 * H].astype(BD),
        f32("proj_b").astype(BD),
        f32("to_out_b").astype(BD),
        f32("mlp_b2").astype(BD),
        f32("ada_b").astype(BD),
    ]
    wimg = np.concatenate(wparts)
    assert wimg.size == CTB, (wimg.size, CTB)

    x = f32("x")
    c = f32("c")
    t_emb = f32("t_emb")
    qkv_b = f32("qkv_b")
    mlp_b1 = f32("mlp_b1")

    in_maps = []
    for b in range(B):
        pf = np.empty(F32TOT, np.float32)
        pf[XO:XO + N * H] = x[b].ravel()
        pf[TEO:TEO + H] = t_emb[b]
        pf[QKVBO:QKVBO + 3 * H] = qkv_b
        pf[B1O:B1O + MLP] = mlp_b1
        cT = np.ascontiguousarray(c[b].T).astype(BD).ravel()
        pb = np.concatenate([wimg, cT])
        assert pb.size == BF16TOT
        in_maps.append({"packed_f32": pf, "packed_bf16": pb})
    return in_maps


_BUILT = None


def _get_built():
    global _BUILT
    if _BUILT is None:
        nc = bass.Bass()
        build(nc)
        _BUILT = nc
    return _BUILT


def kernel(**inputs) -> np.ndarray:
    from concourse.bass_utils import run_bass_kernel_spmd

    nc = _get_built()
    in_maps = make_in_maps(inputs)
    res = run_bass_kernel_spmd(nc, in_maps, list(range(B)))
    return np.stack([res.results[b]["out"] for b in range(B)], axis=0)
